# revision 23
# baseline (speedup 1.0000x reference)
"""CondConv (routing -> per-sample mixed 3x3 conv -> frozen BN -> ReLU -> residual)
on 8 Trainium2 NeuronCores, data-parallel over batch (4 samples/core).

Per core:
  - expert bank resident in SBUF as bf16, host-pretransposed to
    [ct][128cin, oi, tap, e, cout128] (taps in consumption order, center
    first) so the bank arrives in a few large contiguous DMAs, priority
    slices first; all DMA issues on the sync engine (issues block the
    issuing engine's instruction queue)
  - routing: GAP via ACT Copy+accum / DVE reduces overlapped with the x
    DMAs; route_b and the 1/HW mean scale are folded into the dot and
    the sigmoid; sample 0's partition reduce is a deterministic PE
    ones-matmul (gpsimd custom-op has multi-us dispatch jitter), later
    samples use gpsimd off the critical path
  - per-sample mixed kernel: DVE scalar_tensor_tensor accumulation in
    bf16 with ACT-assisted expert scaling; sample 0's oi=0 ct0 half is
    mixed in tap-blocks as expert-pair trees so the first matmuls start
    ~1.3us after the sigmoid
  - conv: per output tile, 18 accumulating bf16 matmuls (2 cin tiles x
    3x3 taps; fp32 PSUM) against width-padded bf16 images; moving dim =
    8 rows x 56 cols = 448. Sample 0 / oi 0 issues all ct0 taps for all
    7 row chunks first (7 open PSUM groups), then the ct1 taps, so the
    PE never waits on the ct1 mix chain; the kernel's final row chunk is
    split in half to shorten the evacuation tail
  - warm-up: dummy matmuls from t=0, with blocks gated on the x DMAs and
    the routing chain so the PE tracks actual preamble progress and the
    HAM clock window never re-throttles, regardless of DMA jitter
  - BN(frozen)+ReLU fused into the ACT PSUM evacuation (BN fold uses a
    DVE Newton rsqrt -- no ACT Sqrt table thrash), residual add on DVE,
    bf16 output upcast to fp32 on host
"""

import threading

import ml_dtypes
import numpy as np

import concourse.bass as bass
import concourse.mybir as mybir
import concourse.tile as tile
from concourse import bacc, bass_isa
from concourse.bass_utils import run_bass_kernel_spmd

F32 = mybir.dt.float32
BF16 = mybir.dt.bfloat16
F8 = mybir.dt.float8e4
AX = mybir.AxisListType
OP = mybir.AluOpType
AF = mybir.ActivationFunctionType

N_CORES = 8
B, CIN, COUT, H, W, KS, E = 32, 256, 256, 56, 56, 3, 4
BPC = B // N_CORES  # samples per core
CT = CIN // 128     # cin partition tiles
OTN = COUT // 128   # cout partition tiles
KK = KS * KS
WP = W + 2          # width zero-padded (kj shifts); height handled by clipping
XO = 1              # column where the real image starts
RC = 7              # row chunks per image
RH = H // RC        # rows per chunk
NF = RH * W         # moving-dim elements per matmul
BN_EPS = 1e-5
NDUM = 38           # warm-up dummy matmuls covering the preamble

# conv taps in consumption order, center first: the center tap covers the
# full output chunk, so it carries start=True and clears every PSUM
# has_written bit; row-clipped taps then accumulate flat sub-slices
# (= 'same' padding semantics at top/bottom). The host lays the weight
# bank out in this same order so tap-block b of the bank is contiguous.
TAPS = [(1, 1)] + [(ki, kj) for ki in range(KS) for kj in range(KS)
                   if (ki, kj) != (1, 1)]
TAP_KKI = [ki * KS + kj for ki, kj in TAPS]
# tap blocks for sample-0 fine-grained mixing (first block holds the
# start=True center tap)
BLKS = [(0, 1), (1, 2), (2, 5), (5, 9)]


def build_bass():
    nc = bacc.Bacc("TRN2", target_bir_lowering=False, debug=False)

    x_d = nc.dram_tensor("x", [BPC, CIN, H, WP], BF16, kind="ExternalInput")
    x8_d = nc.dram_tensor("x8", [CT, 128, H, WP], F8, kind="ExternalInput")
    wt_d = nc.dram_tensor("wt", [CT, 128, OTN, KK, E, 128], BF16,
                          kind="ExternalInput")
    pp_d = nc.dram_tensor("pp", [128, 20], F32, kind="ExternalInput")
    y_d = nc.dram_tensor("y", [BPC, COUT, H, W], BF16, kind="ExternalOutput")

    x_ap = x_d.ap()
    x8_ap = x8_d.ap()
    wt_ap = wt_d.ap()
    pp_ap = pp_d.ap()
    y_ap = y_d.ap()

    with tile.TileContext(nc) as tc:
        with (
            tc.tile_pool(name="wbp", bufs=1) as wbp,
            tc.tile_pool(name="xpp", bufs=1) as xpp,
            tc.tile_pool(name="mwp", bufs=1) as mwp,
            tc.tile_pool(name="otp", bufs=10) as otp,
            tc.tile_pool(name="snp", bufs=1) as snp,
            tc.tile_pool(name="smp", bufs=4) as smp,
            tc.tile_pool(name="psp", bufs=7, space="PSUM") as psp,
        ):
            # ---- persistent tiles ----
            wba = [wbp.tile([128, OTN, KK, E, 128], BF16, name=f"wb{t}",
                            tag=f"wb{t}") for t in range(CT)]
            xp = [[xpp.tile([128, H, WP], BF16, name=f"xp{i}_{t}",
                            tag=f"xp{i}_{t}")
                   for t in range(CT)] for i in range(2)]
            mw = [[mwp.tile([128, OTN, KK, 128], BF16, name=f"mw{i}_{t}",
                            tag=f"mw{i}_{t}")
                   for t in range(CT)] for i in range(2)]
            pp_sb = snp.tile([128, 20], F32, name="pp_sb", tag="pp_sb")
            x8t = [snp.tile([128, H, WP], F8, name=f"x8_{t}", tag=f"x8_{t}")
                   for t in range(CT)]
            bn_inv = [snp.tile([128, 1], F32, name=f"bninv{o}", tag=f"bninv{o}")
                      for o in range(OTN)]
            bn_shift = [snp.tile([128, 1], F32, name=f"bnsh{o}", tag=f"bnsh{o}")
                        for o in range(OTN)]
            zeros1 = snp.tile([128, 1], F32, name="zeros1", tag="zeros1")
            pscr = snp.tile([128, H * WP], BF16, name="pscr", tag="pscr")
            warm_w = snp.tile([128, 128], BF16, name="warm_w", tag="warm_w")
            ones_f = snp.tile([128, 128], F32, name="ones_f", tag="ones_f")
            warm_x = snp.tile([128, NF], BF16, name="warm_x", tag="warm_x")
            tgarb = snp.tile([128, 1], F32, name="tgarb", tag="tgarb")

            # engine-queue preludes: memsets gate the warm-up matmuls; the
            # dummy Sigmoid pulls the ACT function-table load (1.3us) off
            # the routing critical path
            nc.vector.memset(zeros1, 0.0)
            nc.vector.memset(ones_f, 1.0)
            nc.vector.memset(warm_w, 0.0)
            nc.vector.memset(warm_x, 0.0)
            nc.scalar.activation(out=tgarb, in_=zeros1, func=AF.Sigmoid,
                                 bias=zeros1)

            # warm-up: ungated dummy matmuls keep the PE HAM window busy
            # from t=0 so the real stream starts at full clock
            wps = psp.tile([128, 512], F32, name="warm_ps", tag="warmps",
                           bufs=1)
            for _ in range(NDUM):
                nc.tensor.matmul(wps[:, 0:NF], lhsT=warm_w, rhs=warm_x,
                                 start=True, stop=True)
            # gated warm-up blocks: track the actual DMA/routing progress so
            # the PE never idles long enough to trip the HAM MID window,
            # regardless of DMA jitter. Block B gates on the x-t0 DMA,
            # block C on the x-t1 second half.
            def warm_block(n, rhs):
                for _ in range(n):
                    nc.tensor.matmul(wps[:, 0:rhs.shape[1] * rhs.shape[2]],
                                     lhsT=warm_w, rhs=rhs,
                                     start=True, stop=True)

            # ---- preamble DMAs. Issues only on sync/gpsimd: DMA issues (and
            # their sem-lane-recycle waits) block the issuing engine's queue,
            # so scalar/vector must stay clean for routing/mixing. x sample 0
            # in row halves (GAP overlaps the second half's transfer), packed
            # params, then the expert bank with the tap blocks the first
            # conv consumes first.
            nc.sync.dma_start(out=xp[0][0][:, 0:28, :],
                              in_=x_ap[0, 0:128, 0:28, :])
            nc.sync.dma_start(out=xp[0][0][:, 28:56, :],
                              in_=x_ap[0, 0:128, 28:56, :])
            nc.gpsimd.dma_start(out=xp[0][1][:, 0:28, :],
                                in_=x_ap[0, 128:256, 0:28, :])
            nc.gpsimd.dma_start(out=xp[0][1][:, 28:56, :],
                                in_=x_ap[0, 128:256, 28:56, :])
            nc.sync.dma_start(out=pp_sb, in_=pp_ap[:, :])
            for a, b in BLKS:
                nc.sync.dma_start(out=wba[0][:, 0, a:b],
                                  in_=wt_ap[0, :, 0, a:b])
                nc.gpsimd.dma_start(out=wba[1][:, 0, a:b],
                                    in_=wt_ap[1, :, 0, a:b])
            nc.sync.dma_start(out=wba[0][:, 1], in_=wt_ap[0, :, 1])
            nc.sync.dma_start(out=wba[1][:, 1], in_=wt_ap[1, :, 1])

            warm_block(12, x8t[0][:, 0:8, :])
            warm_block(5, x8t[1][:, 28:36, :])

            rwt = [pp_sb[:, 0:4], pp_sb[:, 4:8]]
            rb_bc = pp_sb[:, 8:12]

            def routing(s, chunked, rt_ps=None):
                """GAP -> linear -> sigmoid for sample s. Returns rr [128,E].
                GAP runs on ACT (Copy + accum_out) to keep DVE free."""
                i = s % 2
                pl = [smp.tile([128, 1], F32, name=f"pl{s}_{t}", tag=f"pl{t}")
                      for t in range(CT)]
                if chunked:
                    # GAP over the fp8 shadow: ACT accumulates t0 halves,
                    # DVE reduces t1 halves, pipelined with the DMAs
                    pa = [smp.tile([128, 1], F32, name=f"pa{s}_{c}",
                                   tag=f"pa{c}") for c in range(2)]
                    pb = [smp.tile([128, 1], F32, name=f"pb{s}_{c}",
                                   tag=f"pb{c}") for c in range(2)]
                    for c in range(2):
                        h0 = c * 28
                        nc.scalar.activation(out=pscr[:, :28 * WP],
                                             in_=x8t[0][:, h0:h0 + 28, :],
                                             func=AF.Copy, accum_out=pa[c])
                        nc.vector.reduce_sum(out=pb[c],
                                             in_=x8t[1][:, h0:h0 + 28, :],
                                             axis=AX.XY)
                    nc.vector.tensor_add(pl[0], pa[0], pa[1])
                    nc.vector.tensor_add(pl[1], pb[0], pb[1])
                else:
                    for t in range(CT):
                        nc.scalar.activation(out=pscr[:, :H * WP],
                                             in_=xp[i][t],
                                             func=AF.Copy, accum_out=pl[t])
                prod = smp.tile([128, E], F32, name=f"prod{s}", tag="prod")
                nc.vector.scalar_tensor_tensor(out=prod, in0=rwt[0],
                                               scalar=pl[0], in1=rb_bc,
                                               op0=OP.mult, op1=OP.add)
                nc.vector.scalar_tensor_tensor(out=prod, in0=rwt[1],
                                               scalar=pl[1], in1=prod,
                                               op0=OP.mult, op1=OP.add)
                rr = smp.tile([128, E], F32, name=f"rr{s}", tag="rr")
                if chunked:
                    # sample 0's partition reduce on the PE: ones.T @ prod
                    # sums all 128 partitions and broadcasts the result to
                    # every output partition. Lands in the spare tail of an
                    # acc bank (NOT the warm-up bank) so the filler dummies
                    # that pad the sigmoid+mix window can't collide with
                    # the sigmoid's PSUM read. Deterministic, unlike the
                    # gpsimd custom-op (multi-us dispatch jitter).
                    nc.tensor.matmul(rt_ps[:, NF:NF + E], lhsT=ones_f,
                                     rhs=prod, start=True, stop=True)
                    # short filler dummies keep the PE HAM window busy while
                    # the sigmoid + first mix block complete (a ~2us idle
                    # gap here has been observed to re-throttle the clock)
                    for _ in range(56):
                        nc.tensor.matmul(wps[:, 0:64], lhsT=warm_w,
                                         rhs=warm_x[:, 0:64],
                                         start=True, stop=True)
                    nc.scalar.activation(out=rr, in_=rt_ps[:, NF:NF + E],
                                         func=AF.Sigmoid,
                                         scale=1.0 / (H * W), bias=zeros1)
                else:
                    # later samples have ~13us of slack: gpsimd jitter is
                    # harmless there, and this keeps the mid-stream PE FIFO
                    # free of routing matmuls
                    lg = smp.tile([128, E], F32, name=f"lg{s}", tag="lg")
                    nc.gpsimd.partition_all_reduce(
                        lg, prod, channels=128,
                        reduce_op=bass_isa.ReduceOp.add)
                    nc.scalar.activation(out=rr, in_=lg, func=AF.Sigmoid,
                                         scale=1.0 / (H * W), bias=zeros1)
                return rr

            def mix_dve(s, oi, t, a, b, rr):
                """Accumulate experts into mw[s%2][t][:, oi, a:b] on DVE."""
                i = s % 2
                nc.vector.tensor_scalar_mul(mw[i][t][:, oi, a:b],
                                            wba[t][:, oi, a:b, 0, :],
                                            rr[:, 0:1])
                for e in range(1, E):
                    nc.vector.scalar_tensor_tensor(
                        out=mw[i][t][:, oi, a:b],
                        in0=wba[t][:, oi, a:b, e, :],
                        scalar=rr[:, e:e + 1], in1=mw[i][t][:, oi, a:b],
                        op0=OP.mult, op1=OP.add)

            def mix_act(s, oi, t, rr):
                """Same, with expert scaling on ACT (scaled Copy) and DVE
                doing only the adds, so two mix chains overlap."""
                i = s % 2
                ce = [smp.tile([128, KK, 128], BF16, name=f"ce{s}_{oi}_{e}",
                               tag=f"ce{e}", bufs=2) for e in range(E)]
                for e in range(E):
                    nc.scalar.activation(out=ce[e], in_=wba[t][:, oi, :, e, :],
                                         func=AF.Copy, scale=rr[:, e:e + 1])
                nc.vector.tensor_add(mw[i][t][:, oi], ce[0], ce[1])
                nc.vector.tensor_add(mw[i][t][:, oi], mw[i][t][:, oi], ce[2])
                nc.vector.tensor_add(mw[i][t][:, oi], mw[i][t][:, oi], ce[3])

            def bn_fold():
                # inv = gamma / sqrt(var+eps); shift = beta - mean * inv.
                # rsqrt via linear seed + 2 Newton steps (var is bounded in
                # [0.5, 1.5]) -- pure DVE, no ACT Sqrt = no function-table
                # thrash. The whole chain is gated on the ct1 mix output so
                # the scheduler cannot wedge it into the routing window.
                gate = smp.tile([128, 1], F32, name="bngate", tag="bngate")
                nc.vector.tensor_scalar_mul(gate, mw[0][1][:, 0, KK - 1, 0:1],
                                            0.0)
                for o in range(OTN):
                    p = 12 + 4 * o
                    va = smp.tile([128, 1], F32, name=f"va{o}", tag=f"va{o}")
                    nc.vector.tensor_scalar_add(va, pp_sb[:, p + 3:p + 4],
                                                BN_EPS)
                    ve = smp.tile([128, 1], F32, name=f"ve{o}", tag=f"ve{o}")
                    nc.vector.tensor_add(ve, va, gate)
                    r = bn_inv[o]
                    nc.vector.tensor_scalar(r, ve, -0.5977, 1.6561,
                                            op0=OP.mult, op1=OP.add)
                    t = smp.tile([128, 1], F32, name=f"nt{o}", tag=f"nt{o}")
                    for _ in range(2):
                        nc.vector.tensor_mul(t, r, r)
                        nc.vector.tensor_mul(t, t, ve)
                        nc.vector.tensor_scalar(t, t, -0.5, 1.5,
                                                op0=OP.mult, op1=OP.add)
                        nc.vector.tensor_mul(r, r, t)
                    nc.vector.tensor_mul(bn_inv[o], r, pp_sb[:, p:p + 1])
                    mi = smp.tile([128, 1], F32, name=f"mi{o}", tag=f"mi{o}")
                    nc.vector.tensor_mul(mi, pp_sb[:, p + 2:p + 3], bn_inv[o])
                    nc.vector.tensor_sub(bn_shift[o], pp_sb[:, p + 1:p + 2],
                                         mi)

            def mm(s, oi, t, k, r0, nr, acc, start, stop):
                i = s % 2
                ki, kj = TAPS[k]
                h_lo = max(r0, 1 - ki)
                h_hi = min(r0 + nr - 1, H - ki)
                nc.tensor.matmul(
                    acc[:, (h_lo - r0) * W:(h_hi - r0 + 1) * W],
                    lhsT=mw[i][t][:, oi, k, :],
                    rhs=xp[i][t][:, h_lo + ki - 1:h_hi + ki,
                                 XO - 1 + kj:XO - 1 + kj + W],
                    start=start, stop=stop)

            def evac(s, oi, r0, nr, acc):
                """BN+ReLU on ACT (PSUM read), residual add on DVE (bf16 2x),
                bf16 store."""
                i = s % 2
                ob = otp.tile([128, NF], BF16, name=f"ob{s}_{oi}_{r0}",
                              tag="ob")
                obs = ob[:, :nr * W]
                nc.scalar.activation(out=obs, in_=acc[:, :nr * W],
                                     func=AF.Relu,
                                     bias=bn_shift[oi], scale=bn_inv[oi])
                ob3 = obs.rearrange("p (a b) -> p a b", a=nr)
                nc.vector.tensor_add(ob3, ob3,
                                     xp[i][oi][:, r0:r0 + nr, XO:XO + W])
                nc.sync.dma_start(out=y_ap[s, oi * 128:oi * 128 + 128,
                                           r0:r0 + nr, :],
                                  in_=ob3)

            def conv(s, oi):
                """One output channel tile: row chunks x 18 matmuls each.
                The very last chunk of the kernel is split in half so its
                evacuation tail (BN+residual+DMA) is shorter."""
                chunks = [(rc * RH, RH) for rc in range(RC)]
                if s == BPC - 1 and oi == 1:
                    chunks = chunks[:-1] + [(48, 4), (52, 4)]
                for r0, nr in chunks:
                    acc = psp.tile([128, NF + 4], F32,
                                   name=f"acc{s}_{oi}_{r0}", tag="acc")
                    k = 0
                    for t in range(CT):
                        for kt in range(KK):
                            mm(s, oi, t, kt, r0, nr, acc, start=(k == 0),
                               stop=(k == 2 * KK - 1))
                            k += 1
                    evac(s, oi, r0, nr, acc)

            def conv_sweep(s, oi):
                """First conv: all ct0 taps for all chunks (in tap-block
                order, tracking the fine-grained mix), then the ct1 taps.
                Keeps the PE fed while the ct1 mix chain completes. Uses
                7 PSUM banks (+1 warm-up) = all 8."""
                accs = acc0
                for a, b in BLKS:
                    for rc in range(RC):
                        for kt in range(a, b):
                            mm(s, oi, 0, kt, rc * RH, RH, accs[rc],
                               start=(kt == 0), stop=False)
                for rc in range(RC):
                    for kt in range(KK):
                        mm(s, oi, 1, kt, rc * RH, RH, accs[rc], start=False,
                           stop=(kt == KK - 1))
                for rc in range(RC):
                    evac(s, oi, rc * RH, RH, accs[rc])

            def mix_tree(s, oi, t, a, b, rr):
                """Expert-pair tree mix: two independent 2-deep chains plus
                a final add -- lower latency than the 4-deep stt chain."""
                i = s % 2
                c1 = smp.tile([128, b - a, 128], BF16,
                              name=f"mc1_{s}_{oi}_{a}", tag=f"mc1_{a}")
                c2 = smp.tile([128, b - a, 128], BF16,
                              name=f"mc2_{s}_{oi}_{a}", tag=f"mc2_{a}")
                nc.vector.tensor_scalar_mul(c1, wba[t][:, oi, a:b, 0, :],
                                            rr[:, 0:1])
                nc.vector.tensor_scalar_mul(c2, wba[t][:, oi, a:b, 2, :],
                                            rr[:, 2:3])
                nc.vector.scalar_tensor_tensor(
                    out=c1, in0=wba[t][:, oi, a:b, 1, :],
                    scalar=rr[:, 1:2], in1=c1, op0=OP.mult, op1=OP.add)
                nc.vector.scalar_tensor_tensor(
                    out=c2, in0=wba[t][:, oi, a:b, 3, :],
                    scalar=rr[:, 3:4], in1=c2, op0=OP.mult, op1=OP.add)
                nc.vector.tensor_add(mw[i][t][:, oi, a:b], c1, c2)

            # ---- program ----
            acc0 = [psp.tile([128, NF + 4], F32, name=f"acc0_0_{rc}",
                             tag="acc") for rc in range(RC)]
            rr0 = routing(0, chunked=True, rt_ps=acc0[6])
            # sample 0, oi 0: fine-grained tap-block mixing on DVE; ct1
            # via ACT scaled copies (the ct0 sweep buys ~12us of slack and
            # ACT is idle after the sigmoid)
            for a, b in BLKS:
                mix_tree(0, 0, 0, a, b, rr0)
            mix_act(0, 0, 1, rr0)
            bn_fold()
            rrs = {0: rr0}

            for s in range(BPC):
                if s + 1 < BPC:
                    nc.sync.dma_start(out=xp[(s + 1) % 2][0],
                                      in_=x_ap[s + 1, 0:128, :, :])
                    nc.gpsimd.dma_start(out=xp[(s + 1) % 2][1],
                                        in_=x_ap[s + 1, 128:256, :, :])
                if s == 0:
                    conv_sweep(0, 0)
                    # oi=1 mixing for sample 0 (ACT-assisted on ct1)
                    mix_dve(0, 1, 0, 0, KK, rr0)
                    mix_act(0, 1, 1, rr0)
                else:
                    conv(s, 0)
                if s + 1 < BPC:
                    rr = routing(s + 1, chunked=False)
                    rrs[s + 1] = rr
                    mix_dve(s + 1, 0, 0, 0, KK, rr)
                    mix_act(s + 1, 0, 1, rr)
                conv(s, 1)
                if s + 1 < BPC:
                    rr = rrs[s + 1]
                    mix_dve(s + 1, 1, 0, 0, KK, rr)
                    mix_act(s + 1, 1, 1, rr)

    nc.compile()
    return nc


_CACHE = {}
_LOCK = threading.Lock()


def prepare_in_maps(inputs):
    """Host-side layout prep (sharding + transposes + dtype casts only)."""
    x = np.asarray(inputs["x"], dtype=np.float32)
    route_w = np.asarray(inputs["route_w"], dtype=np.float32)
    route_b = np.asarray(inputs["route_b"], dtype=np.float32)
    expert_w = np.asarray(inputs["expert_w"], dtype=np.float32)
    bn_gamma = np.asarray(inputs["bn_gamma"], dtype=np.float32)
    bn_beta = np.asarray(inputs["bn_beta"], dtype=np.float32)
    bn_mean = np.asarray(inputs["bn_mean"], dtype=np.float32)
    bn_var = np.asarray(inputs["bn_var"], dtype=np.float32)

    # [E, COUT, CIN, K, K] -> [CT, 128cin, OTN, tap(consumption order), E,
    # 128cout]
    a = expert_w.reshape(E, OTN, 128, CIN, KS, KS)
    b = a.transpose(3, 1, 4, 5, 0, 2).reshape(CIN, OTN, KK, E, 128)
    b = b[:, :, TAP_KKI]
    wt = np.ascontiguousarray(b.reshape(CT, 128, OTN, KK, E, 128)).astype(
        ml_dtypes.bfloat16)

    # packed params [128, 20]: rwt t0 | rwt t1 | rb (replicated) | bn o0 |
    # bn o1 (gamma, beta, mean, var columns)
    rwt = route_w.T  # [CIN, E]
    bnp = np.stack([bn_gamma, bn_beta, bn_mean, bn_var], axis=1)  # [COUT, 4]
    # rb is folded into the pre-allreduce dot: each of the 128 partitions
    # contributes rb*H*W/128, and the sigmoid applies the 1/(H*W) scale
    rb_fold = route_b * (H * W) / 128.0
    pp = np.concatenate([rwt[0:128], rwt[128:256],
                         np.tile(rb_fold[None, :], (128, 1)),
                         bnp[0:128], bnp[128:256]], axis=1)
    pp = np.ascontiguousarray(pp.astype(np.float32))

    # width-pad on host: [0, halo, x, halo, 0] -> residual read of the
    # image columns starts 4B-aligned; border halos arrive pre-zeroed
    xpad = np.zeros((B, CIN, H, WP), dtype=ml_dtypes.bfloat16)
    xpad[:, :, :, XO:XO + W] = x.astype(ml_dtypes.bfloat16)

    x8 = xpad[:, :, :, :].astype(ml_dtypes.float8_e4m3fn)
    return [
        {"x": np.ascontiguousarray(xpad[c * BPC:(c + 1) * BPC]),
         "x8": np.ascontiguousarray(
             x8[c * BPC].reshape(CT, 128, H, WP)),
         "wt": wt, "pp": pp}
        for c in range(N_CORES)
    ]


def _get_nc():
    with _LOCK:
        if "nc" not in _CACHE:
            _CACHE["nc"] = build_bass()
        return _CACHE["nc"]


def kernel(**inputs):
    in_maps = prepare_in_maps(inputs)
    nc = _get_nc()
    res = run_bass_kernel_spmd(nc, in_maps, core_ids=list(range(N_CORES)))
    return np.concatenate([np.asarray(r["y"], dtype=np.float32)
                           for r in res.results], axis=0)


# revision 24
# speedup vs baseline: 1.0068x; 1.0068x over previous
"""CondConv (routing -> per-sample mixed 3x3 conv -> frozen BN -> ReLU -> residual)
on 8 Trainium2 NeuronCores, data-parallel over batch (4 samples/core).

Per core:
  - expert bank resident in SBUF as bf16, host-pretransposed to
    [ct][128cin, oi, tap, e, cout128] (taps in consumption order, center
    first) so the bank arrives in a few large contiguous DMAs, priority
    slices first; all DMA issues on the sync engine (issues block the
    issuing engine's instruction queue)
  - routing: GAP via ACT Copy+accum / DVE reduces overlapped with the x
    DMAs; route_b and the 1/HW mean scale are folded into the dot and
    the sigmoid; sample 0's partition reduce is a deterministic PE
    ones-matmul (gpsimd custom-op has multi-us dispatch jitter), later
    samples use gpsimd off the critical path
  - per-sample mixed kernel: DVE scalar_tensor_tensor accumulation in
    bf16 with ACT-assisted expert scaling; sample 0's oi=0 ct0 half is
    mixed in tap-blocks as expert-pair trees so the first matmuls start
    ~1.3us after the sigmoid
  - conv: per output tile, 18 accumulating bf16 matmuls (2 cin tiles x
    3x3 taps; fp32 PSUM) against width-padded bf16 images; moving dim =
    8 rows x 56 cols = 448. Sample 0 / oi 0 issues all ct0 taps for all
    7 row chunks first (7 open PSUM groups), then the ct1 taps, so the
    PE never waits on the ct1 mix chain; the kernel's final row chunk is
    split in half to shorten the evacuation tail
  - warm-up: dummy matmuls from t=0, with blocks gated on the x DMAs and
    the routing chain so the PE tracks actual preamble progress and the
    HAM clock window never re-throttles, regardless of DMA jitter
  - BN(frozen)+ReLU fused into the ACT PSUM evacuation (BN fold uses a
    DVE Newton rsqrt -- no ACT Sqrt table thrash), residual add on DVE,
    bf16 output upcast to fp32 on host
"""

import threading

import ml_dtypes
import numpy as np

import concourse.bass as bass
import concourse.mybir as mybir
import concourse.tile as tile
from concourse import bacc, bass_isa
from concourse.bass_utils import run_bass_kernel_spmd

F32 = mybir.dt.float32
BF16 = mybir.dt.bfloat16
AX = mybir.AxisListType
OP = mybir.AluOpType
AF = mybir.ActivationFunctionType

N_CORES = 8
B, CIN, COUT, H, W, KS, E = 32, 256, 256, 56, 56, 3, 4
BPC = B // N_CORES  # samples per core
CT = CIN // 128     # cin partition tiles
OTN = COUT // 128   # cout partition tiles
KK = KS * KS
WP = W + 2          # width zero-padded (kj shifts); height handled by clipping
XO = 1              # column where the real image starts
RC = 7              # row chunks per image
RH = H // RC        # rows per chunk
NF = RH * W         # moving-dim elements per matmul
BN_EPS = 1e-5
NDUM = 38           # warm-up dummy matmuls covering the preamble

# conv taps in consumption order, center first: the center tap covers the
# full output chunk, so it carries start=True and clears every PSUM
# has_written bit; row-clipped taps then accumulate flat sub-slices
# (= 'same' padding semantics at top/bottom). The host lays the weight
# bank out in this same order so tap-block b of the bank is contiguous.
TAPS = [(1, 1)] + [(ki, kj) for ki in range(KS) for kj in range(KS)
                   if (ki, kj) != (1, 1)]
TAP_KKI = [ki * KS + kj for ki, kj in TAPS]
# tap blocks for sample-0 fine-grained mixing (first block holds the
# start=True center tap)
BLKS = [(0, 1), (1, 2), (2, 5), (5, 9)]


def build_bass():
    nc = bacc.Bacc("TRN2", target_bir_lowering=False, debug=False)

    x_d = nc.dram_tensor("x", [BPC, CIN, H, WP], BF16, kind="ExternalInput")
    wt_d = nc.dram_tensor("wt", [CT, 128, OTN, KK, E, 128], BF16,
                          kind="ExternalInput")
    pp_d = nc.dram_tensor("pp", [128, 20], F32, kind="ExternalInput")
    y_d = nc.dram_tensor("y", [BPC, COUT, H, W], BF16, kind="ExternalOutput")

    x_ap = x_d.ap()
    wt_ap = wt_d.ap()
    pp_ap = pp_d.ap()
    y_ap = y_d.ap()

    with tile.TileContext(nc) as tc:
        with (
            tc.tile_pool(name="wbp", bufs=1) as wbp,
            tc.tile_pool(name="xpp", bufs=1) as xpp,
            tc.tile_pool(name="mwp", bufs=1) as mwp,
            tc.tile_pool(name="otp", bufs=10) as otp,
            tc.tile_pool(name="snp", bufs=1) as snp,
            tc.tile_pool(name="smp", bufs=4) as smp,
            tc.tile_pool(name="psp", bufs=7, space="PSUM") as psp,
        ):
            # ---- persistent tiles ----
            wba = [wbp.tile([128, OTN, KK, E, 128], BF16, name=f"wb{t}",
                            tag=f"wb{t}") for t in range(CT)]
            xp = [[xpp.tile([128, H, WP], BF16, name=f"xp{i}_{t}",
                            tag=f"xp{i}_{t}")
                   for t in range(CT)] for i in range(2)]
            mw = [[mwp.tile([128, OTN, KK, 128], BF16, name=f"mw{i}_{t}",
                            tag=f"mw{i}_{t}")
                   for t in range(CT)] for i in range(2)]
            pp_sb = snp.tile([128, 20], F32, name="pp_sb", tag="pp_sb")
            bn_inv = [snp.tile([128, 1], F32, name=f"bninv{o}", tag=f"bninv{o}")
                      for o in range(OTN)]
            bn_shift = [snp.tile([128, 1], F32, name=f"bnsh{o}", tag=f"bnsh{o}")
                        for o in range(OTN)]
            zeros1 = snp.tile([128, 1], F32, name="zeros1", tag="zeros1")
            pscr = snp.tile([128, H * WP], BF16, name="pscr", tag="pscr")
            warm_w = snp.tile([128, 128], BF16, name="warm_w", tag="warm_w")
            ones_f = snp.tile([128, 128], F32, name="ones_f", tag="ones_f")
            warm_x = snp.tile([128, NF], BF16, name="warm_x", tag="warm_x")
            tgarb = snp.tile([128, 1], F32, name="tgarb", tag="tgarb")

            # engine-queue preludes: memsets gate the warm-up matmuls; the
            # dummy Sigmoid pulls the ACT function-table load (1.3us) off
            # the routing critical path
            nc.vector.memset(zeros1, 0.0)
            nc.vector.memset(ones_f, 1.0)
            nc.vector.memset(warm_w, 0.0)
            nc.vector.memset(warm_x, 0.0)
            nc.scalar.activation(out=tgarb, in_=zeros1, func=AF.Sigmoid,
                                 bias=zeros1)

            # warm-up: ungated dummy matmuls keep the PE HAM window busy
            # from t=0 so the real stream starts at full clock
            wps = psp.tile([128, 512], F32, name="warm_ps", tag="warmps",
                           bufs=1)
            for _ in range(NDUM):
                nc.tensor.matmul(wps[:, 0:NF], lhsT=warm_w, rhs=warm_x,
                                 start=True, stop=True)
            # gated warm-up blocks: track the actual DMA/routing progress so
            # the PE never idles long enough to trip the HAM MID window,
            # regardless of DMA jitter. Block B gates on the x-t0 DMA,
            # block C on the x-t1 second half.
            def warm_block(n, rhs):
                for _ in range(n):
                    nc.tensor.matmul(wps[:, 0:rhs.shape[1] * rhs.shape[2]],
                                     lhsT=warm_w, rhs=rhs,
                                     start=True, stop=True)

            # ---- preamble DMAs. Issues only on sync/gpsimd: DMA issues (and
            # their sem-lane-recycle waits) block the issuing engine's queue,
            # so scalar/vector must stay clean for routing/mixing. x sample 0
            # in row halves (GAP overlaps the second half's transfer), packed
            # params, then the expert bank with the tap blocks the first
            # conv consumes first.
            nc.sync.dma_start(out=xp[0][0][:, 0:28, :],
                              in_=x_ap[0, 0:128, 0:28, :])
            nc.sync.dma_start(out=xp[0][0][:, 28:56, :],
                              in_=x_ap[0, 0:128, 28:56, :])
            nc.gpsimd.dma_start(out=xp[0][1][:, 0:28, :],
                                in_=x_ap[0, 128:256, 0:28, :])
            nc.gpsimd.dma_start(out=xp[0][1][:, 28:56, :],
                                in_=x_ap[0, 128:256, 28:56, :])
            nc.sync.dma_start(out=pp_sb, in_=pp_ap[:, :])
            for a, b in BLKS:
                nc.sync.dma_start(out=wba[0][:, 0, a:b],
                                  in_=wt_ap[0, :, 0, a:b])
                nc.gpsimd.dma_start(out=wba[1][:, 0, a:b],
                                    in_=wt_ap[1, :, 0, a:b])
            nc.sync.dma_start(out=wba[0][:, 1], in_=wt_ap[0, :, 1])
            nc.sync.dma_start(out=wba[1][:, 1], in_=wt_ap[1, :, 1])

            warm_block(20, xp[0][0][:, 0:8, :])
            warm_block(8, xp[0][1][:, 28:36, :])

            rwt = [pp_sb[:, 0:4], pp_sb[:, 4:8]]
            rb_bc = pp_sb[:, 8:12]

            def routing(s, chunked, rt_ps=None):
                """GAP -> linear -> sigmoid for sample s. Returns rr [128,E].
                GAP runs on ACT (Copy + accum_out) to keep DVE free."""
                i = s % 2
                pl = [smp.tile([128, 1], F32, name=f"pl{s}_{t}", tag=f"pl{t}")
                      for t in range(CT)]
                if chunked:
                    # t0 fully on ACT; t1 arrives in row halves, each
                    # reduced on DVE as it lands
                    nc.scalar.activation(out=pscr[:, :H * WP], in_=xp[i][0],
                                         func=AF.Copy, accum_out=pl[0])
                    pa = smp.tile([128, 1], F32, name=f"pa{s}", tag="pa")
                    pb = smp.tile([128, 1], F32, name=f"pb{s}", tag="pb")
                    pc = smp.tile([128, 1], F32, name=f"pc{s}", tag="pc")
                    nc.vector.reduce_sum(out=pa, in_=xp[i][1][:, 0:28, :],
                                         axis=AX.XY)
                    nc.scalar.activation(out=pscr[:, :14 * WP],
                                         in_=xp[i][1][:, 28:42, :],
                                         func=AF.Copy, accum_out=pb)
                    nc.vector.reduce_sum(out=pc, in_=xp[i][1][:, 42:56, :],
                                         axis=AX.XY)
                    nc.vector.tensor_add(pl[1], pa, pb)
                    nc.vector.tensor_add(pl[1], pl[1], pc)
                else:
                    for t in range(CT):
                        nc.scalar.activation(out=pscr[:, :H * WP],
                                             in_=xp[i][t],
                                             func=AF.Copy, accum_out=pl[t])
                prod = smp.tile([128, E], F32, name=f"prod{s}", tag="prod")
                nc.vector.scalar_tensor_tensor(out=prod, in0=rwt[0],
                                               scalar=pl[0], in1=rb_bc,
                                               op0=OP.mult, op1=OP.add)
                nc.vector.scalar_tensor_tensor(out=prod, in0=rwt[1],
                                               scalar=pl[1], in1=prod,
                                               op0=OP.mult, op1=OP.add)
                rr = smp.tile([128, E], F32, name=f"rr{s}", tag="rr")
                if chunked:
                    # sample 0's partition reduce on the PE: ones.T @ prod
                    # sums all 128 partitions and broadcasts the result to
                    # every output partition. Lands in the spare tail of an
                    # acc bank (NOT the warm-up bank) so the filler dummies
                    # that pad the sigmoid+mix window can't collide with
                    # the sigmoid's PSUM read. Deterministic, unlike the
                    # gpsimd custom-op (multi-us dispatch jitter).
                    nc.tensor.matmul(rt_ps[:, NF:NF + E], lhsT=ones_f,
                                     rhs=prod, start=True, stop=True)
                    # short filler dummies keep the PE HAM window busy while
                    # the sigmoid + first mix block complete (a ~2us idle
                    # gap here has been observed to re-throttle the clock)
                    for _ in range(56):
                        nc.tensor.matmul(wps[:, 0:64], lhsT=warm_w,
                                         rhs=warm_x[:, 0:64],
                                         start=True, stop=True)
                    nc.scalar.activation(out=rr, in_=rt_ps[:, NF:NF + E],
                                         func=AF.Sigmoid,
                                         scale=1.0 / (H * W), bias=zeros1)
                else:
                    # later samples have ~13us of slack: gpsimd jitter is
                    # harmless there, and this keeps the mid-stream PE FIFO
                    # free of routing matmuls
                    lg = smp.tile([128, E], F32, name=f"lg{s}", tag="lg")
                    nc.gpsimd.partition_all_reduce(
                        lg, prod, channels=128,
                        reduce_op=bass_isa.ReduceOp.add)
                    nc.scalar.activation(out=rr, in_=lg, func=AF.Sigmoid,
                                         scale=1.0 / (H * W), bias=zeros1)
                return rr

            def mix_dve(s, oi, t, a, b, rr):
                """Accumulate experts into mw[s%2][t][:, oi, a:b] on DVE."""
                i = s % 2
                nc.vector.tensor_scalar_mul(mw[i][t][:, oi, a:b],
                                            wba[t][:, oi, a:b, 0, :],
                                            rr[:, 0:1])
                for e in range(1, E):
                    nc.vector.scalar_tensor_tensor(
                        out=mw[i][t][:, oi, a:b],
                        in0=wba[t][:, oi, a:b, e, :],
                        scalar=rr[:, e:e + 1], in1=mw[i][t][:, oi, a:b],
                        op0=OP.mult, op1=OP.add)

            def mix_act(s, oi, t, rr):
                """Same, with expert scaling on ACT (scaled Copy) and DVE
                doing only the adds, so two mix chains overlap."""
                i = s % 2
                ce = [smp.tile([128, KK, 128], BF16, name=f"ce{s}_{oi}_{e}",
                               tag=f"ce{e}", bufs=2) for e in range(E)]
                for e in range(E):
                    nc.scalar.activation(out=ce[e], in_=wba[t][:, oi, :, e, :],
                                         func=AF.Copy, scale=rr[:, e:e + 1])
                nc.vector.tensor_add(mw[i][t][:, oi], ce[0], ce[1])
                nc.vector.tensor_add(mw[i][t][:, oi], mw[i][t][:, oi], ce[2])
                nc.vector.tensor_add(mw[i][t][:, oi], mw[i][t][:, oi], ce[3])

            def bn_fold():
                # inv = gamma / sqrt(var+eps); shift = beta - mean * inv.
                # rsqrt via linear seed + 2 Newton steps (var is bounded in
                # [0.5, 1.5]) -- pure DVE, no ACT Sqrt = no function-table
                # thrash. The whole chain is gated on the ct1 mix output so
                # the scheduler cannot wedge it into the routing window.
                gate = smp.tile([128, 1], F32, name="bngate", tag="bngate")
                nc.vector.tensor_scalar_mul(gate, mw[0][1][:, 0, KK - 1, 0:1],
                                            0.0)
                for o in range(OTN):
                    p = 12 + 4 * o
                    va = smp.tile([128, 1], F32, name=f"va{o}", tag=f"va{o}")
                    nc.vector.tensor_scalar_add(va, pp_sb[:, p + 3:p + 4],
                                                BN_EPS)
                    ve = smp.tile([128, 1], F32, name=f"ve{o}", tag=f"ve{o}")
                    nc.vector.tensor_add(ve, va, gate)
                    r = bn_inv[o]
                    nc.vector.tensor_scalar(r, ve, -0.5977, 1.6561,
                                            op0=OP.mult, op1=OP.add)
                    t = smp.tile([128, 1], F32, name=f"nt{o}", tag=f"nt{o}")
                    for _ in range(2):
                        nc.vector.tensor_mul(t, r, r)
                        nc.vector.tensor_mul(t, t, ve)
                        nc.vector.tensor_scalar(t, t, -0.5, 1.5,
                                                op0=OP.mult, op1=OP.add)
                        nc.vector.tensor_mul(r, r, t)
                    nc.vector.tensor_mul(bn_inv[o], r, pp_sb[:, p:p + 1])
                    mi = smp.tile([128, 1], F32, name=f"mi{o}", tag=f"mi{o}")
                    nc.vector.tensor_mul(mi, pp_sb[:, p + 2:p + 3], bn_inv[o])
                    nc.vector.tensor_sub(bn_shift[o], pp_sb[:, p + 1:p + 2],
                                         mi)

            def mm(s, oi, t, k, r0, nr, acc, start, stop):
                i = s % 2
                ki, kj = TAPS[k]
                h_lo = max(r0, 1 - ki)
                h_hi = min(r0 + nr - 1, H - ki)
                nc.tensor.matmul(
                    acc[:, (h_lo - r0) * W:(h_hi - r0 + 1) * W],
                    lhsT=mw[i][t][:, oi, k, :],
                    rhs=xp[i][t][:, h_lo + ki - 1:h_hi + ki,
                                 XO - 1 + kj:XO - 1 + kj + W],
                    start=start, stop=stop)

            def evac(s, oi, r0, nr, acc):
                """BN+ReLU on ACT (PSUM read), residual add on DVE (bf16 2x),
                bf16 store."""
                i = s % 2
                ob = otp.tile([128, NF], BF16, name=f"ob{s}_{oi}_{r0}",
                              tag="ob")
                obs = ob[:, :nr * W]
                nc.scalar.activation(out=obs, in_=acc[:, :nr * W],
                                     func=AF.Relu,
                                     bias=bn_shift[oi], scale=bn_inv[oi])
                ob3 = obs.rearrange("p (a b) -> p a b", a=nr)
                nc.vector.tensor_add(ob3, ob3,
                                     xp[i][oi][:, r0:r0 + nr, XO:XO + W])
                nc.sync.dma_start(out=y_ap[s, oi * 128:oi * 128 + 128,
                                           r0:r0 + nr, :],
                                  in_=ob3)

            def conv(s, oi):
                """One output channel tile: row chunks x 18 matmuls each.
                The very last chunk of the kernel is split in half so its
                evacuation tail (BN+residual+DMA) is shorter."""
                chunks = [(rc * RH, RH) for rc in range(RC)]
                if s == BPC - 1 and oi == 1:
                    chunks = chunks[:-1] + [(48, 4), (52, 4)]
                for r0, nr in chunks:
                    acc = psp.tile([128, NF + 4], F32,
                                   name=f"acc{s}_{oi}_{r0}", tag="acc")
                    k = 0
                    for t in range(CT):
                        for kt in range(KK):
                            mm(s, oi, t, kt, r0, nr, acc, start=(k == 0),
                               stop=(k == 2 * KK - 1))
                            k += 1
                    evac(s, oi, r0, nr, acc)

            def conv_sweep(s, oi):
                """First conv: all ct0 taps for all chunks (in tap-block
                order, tracking the fine-grained mix), then the ct1 taps.
                Keeps the PE fed while the ct1 mix chain completes. Uses
                7 PSUM banks (+1 warm-up) = all 8."""
                accs = acc0
                for a, b in BLKS:
                    for rc in range(RC):
                        for kt in range(a, b):
                            mm(s, oi, 0, kt, rc * RH, RH, accs[rc],
                               start=(kt == 0), stop=False)
                for rc in range(RC):
                    for kt in range(KK):
                        mm(s, oi, 1, kt, rc * RH, RH, accs[rc], start=False,
                           stop=(kt == KK - 1))
                for rc in range(RC):
                    evac(s, oi, rc * RH, RH, accs[rc])

            def mix_tree(s, oi, t, a, b, rr):
                """Expert-pair tree mix: two independent 2-deep chains plus
                a final add -- lower latency than the 4-deep stt chain."""
                i = s % 2
                c1 = smp.tile([128, b - a, 128], BF16,
                              name=f"mc1_{s}_{oi}_{a}", tag=f"mc1_{a}")
                c2 = smp.tile([128, b - a, 128], BF16,
                              name=f"mc2_{s}_{oi}_{a}", tag=f"mc2_{a}")
                nc.vector.tensor_scalar_mul(c1, wba[t][:, oi, a:b, 0, :],
                                            rr[:, 0:1])
                nc.vector.tensor_scalar_mul(c2, wba[t][:, oi, a:b, 2, :],
                                            rr[:, 2:3])
                nc.vector.scalar_tensor_tensor(
                    out=c1, in0=wba[t][:, oi, a:b, 1, :],
                    scalar=rr[:, 1:2], in1=c1, op0=OP.mult, op1=OP.add)
                nc.vector.scalar_tensor_tensor(
                    out=c2, in0=wba[t][:, oi, a:b, 3, :],
                    scalar=rr[:, 3:4], in1=c2, op0=OP.mult, op1=OP.add)
                nc.vector.tensor_add(mw[i][t][:, oi, a:b], c1, c2)

            # ---- program ----
            acc0 = [psp.tile([128, NF + 4], F32, name=f"acc0_0_{rc}",
                             tag="acc") for rc in range(RC)]
            rr0 = routing(0, chunked=True, rt_ps=acc0[6])
            # sample 0, oi 0: fine-grained tap-block mixing on DVE; ct1
            # via ACT scaled copies (the ct0 sweep buys ~12us of slack and
            # ACT is idle after the sigmoid)
            for a, b in BLKS:
                mix_tree(0, 0, 0, a, b, rr0)
            mix_act(0, 0, 1, rr0)
            bn_fold()
            rrs = {0: rr0}

            for s in range(BPC):
                if s + 1 < BPC:
                    nc.sync.dma_start(out=xp[(s + 1) % 2][0],
                                      in_=x_ap[s + 1, 0:128, :, :])
                    nc.gpsimd.dma_start(out=xp[(s + 1) % 2][1],
                                        in_=x_ap[s + 1, 128:256, :, :])
                if s == 0:
                    conv_sweep(0, 0)
                    # oi=1 mixing for sample 0 (ACT-assisted on ct1)
                    mix_dve(0, 1, 0, 0, KK, rr0)
                    mix_act(0, 1, 1, rr0)
                else:
                    conv(s, 0)
                if s + 1 < BPC:
                    rr = routing(s + 1, chunked=False)
                    rrs[s + 1] = rr
                    mix_dve(s + 1, 0, 0, 0, KK, rr)
                    mix_act(s + 1, 0, 1, rr)
                conv(s, 1)
                if s + 1 < BPC:
                    rr = rrs[s + 1]
                    mix_dve(s + 1, 1, 0, 0, KK, rr)
                    mix_act(s + 1, 1, 1, rr)

    nc.compile()
    return nc


_CACHE = {}
_LOCK = threading.Lock()


def prepare_in_maps(inputs):
    """Host-side layout prep (sharding + transposes + dtype casts only)."""
    x = np.asarray(inputs["x"], dtype=np.float32)
    route_w = np.asarray(inputs["route_w"], dtype=np.float32)
    route_b = np.asarray(inputs["route_b"], dtype=np.float32)
    expert_w = np.asarray(inputs["expert_w"], dtype=np.float32)
    bn_gamma = np.asarray(inputs["bn_gamma"], dtype=np.float32)
    bn_beta = np.asarray(inputs["bn_beta"], dtype=np.float32)
    bn_mean = np.asarray(inputs["bn_mean"], dtype=np.float32)
    bn_var = np.asarray(inputs["bn_var"], dtype=np.float32)

    # [E, COUT, CIN, K, K] -> [CT, 128cin, OTN, tap(consumption order), E,
    # 128cout]
    a = expert_w.reshape(E, OTN, 128, CIN, KS, KS)
    b = a.transpose(3, 1, 4, 5, 0, 2).reshape(CIN, OTN, KK, E, 128)
    b = b[:, :, TAP_KKI]
    wt = np.ascontiguousarray(b.reshape(CT, 128, OTN, KK, E, 128)).astype(
        ml_dtypes.bfloat16)

    # packed params [128, 20]: rwt t0 | rwt t1 | rb (replicated) | bn o0 |
    # bn o1 (gamma, beta, mean, var columns)
    rwt = route_w.T  # [CIN, E]
    bnp = np.stack([bn_gamma, bn_beta, bn_mean, bn_var], axis=1)  # [COUT, 4]
    # rb is folded into the pre-allreduce dot: each of the 128 partitions
    # contributes rb*H*W/128, and the sigmoid applies the 1/(H*W) scale
    rb_fold = route_b * (H * W) / 128.0
    pp = np.concatenate([rwt[0:128], rwt[128:256],
                         np.tile(rb_fold[None, :], (128, 1)),
                         bnp[0:128], bnp[128:256]], axis=1)
    pp = np.ascontiguousarray(pp.astype(np.float32))

    # width-pad on host: [0, halo, x, halo, 0] -> residual read of the
    # image columns starts 4B-aligned; border halos arrive pre-zeroed
    xpad = np.zeros((B, CIN, H, WP), dtype=ml_dtypes.bfloat16)
    xpad[:, :, :, XO:XO + W] = x.astype(ml_dtypes.bfloat16)

    return [
        {"x": np.ascontiguousarray(xpad[c * BPC:(c + 1) * BPC]),
         "wt": wt, "pp": pp}
        for c in range(N_CORES)
    ]


def _get_nc():
    with _LOCK:
        if "nc" not in _CACHE:
            _CACHE["nc"] = build_bass()
        return _CACHE["nc"]


def kernel(**inputs):
    in_maps = prepare_in_maps(inputs)
    nc = _get_nc()
    res = run_bass_kernel_spmd(nc, in_maps, core_ids=list(range(N_CORES)))
    return np.concatenate([np.asarray(r["y"], dtype=np.float32)
                           for r in res.results], axis=0)
